# revision 2
# baseline (speedup 1.0000x reference)
"""Trainium2 Bass kernel for a dense MoE layer.

Reference computation (all experts run on every token):
    gate = softmax(x @ gate_w + gate_b)                       # [N, E]
    expert_out[e] = x @ expert_w[e] + expert_b[e]             # [E, N, R]
    out = einsum('ne,enr->nr', gate, expert_out)              # [N, R]

Sharding: output-column parallel across 8 cores. Core c computes
out[:, c*512:(c+1)*512] using the full x and the column slice of every
expert's weights. No collectives — the host concatenates the slices.

Device program (per core, SPMD identical, data differs):
  - x arrives pre-transposed (host) as xt[d_in, n_tok] so k-chunks load
    straight into matmul lhsT layout [K=128, M=128 tokens].
  - Loop over token blocks of TB tokens; x block resident in SBUF,
    expert weights stream (N_TOK/TB passes over the 32MB weight slice).
  - Gate: PE matmul accumulation into PSUM [128, E], bias added via a
    K=1 matmul against a ones vector, then max/exp/sum/normalize
    (softmax) on DVE+ACT.
  - Experts: for each expert, 32 K-chunk matmuls accumulate x @ W_e
    into a PSUM bank per token tile; expert bias added via K=1 matmul;
    the gate-weighted combine is one fused DVE op per (tile, expert):
    acc = (psum * gate[:, e]) + acc.
"""

import numpy as np
import ml_dtypes

import concourse.bass as bass  # noqa: F401  (registers rust bindings)
import concourse.mybir as mybir
import concourse.tile as tile
from concourse import bacc
from concourse.bass_utils import run_bass_kernel_spmd
from concourse.bass_interp import get_hw_module

N_CORES = 8
N_TOK, D_IN, D_OUT, E = 4096, 4096, 4096, 8
COLS = D_OUT // N_CORES  # 512 output columns per core
P = 128
TB = 1024  # tokens per block resident in SBUF

F32 = mybir.dt.float32


def build_moe_program(
    n_tok=N_TOK,
    d_in=D_IN,
    cols=COLS,
    e=E,
    tb=TB,
    dtype=mybir.dt.bfloat16,
):
    assert n_tok % tb == 0 and tb % P == 0 and d_in % P == 0
    kchunks = d_in // P
    tpb = tb // P  # token tiles per block
    nblocks = n_tok // tb

    nc = bacc.Bacc("TRN2", target_bir_lowering=False, debug=False)

    xt_d = nc.dram_tensor("xt", [d_in, n_tok], dtype, kind="ExternalInput")
    wc_d = nc.dram_tensor("wc", [e, d_in, cols], dtype, kind="ExternalInput")
    gw_d = nc.dram_tensor("gw", [d_in, e], dtype, kind="ExternalInput")
    gb_d = nc.dram_tensor("gb", [1, e], dtype, kind="ExternalInput")
    eb_d = nc.dram_tensor("eb", [1, e, cols], dtype, kind="ExternalInput")
    out_d = nc.dram_tensor("out", [n_tok, cols], F32, kind="ExternalOutput")

    with tile.TileContext(nc) as tc:
        with (
            tc.tile_pool(name="const", bufs=1) as constp,
            tc.tile_pool(name="xp", bufs=2 * kchunks) as xpool,
            tc.tile_pool(name="wp", bufs=6) as wpool,
            tc.tile_pool(name="accp", bufs=2 * tpb) as accpool,
            tc.tile_pool(name="gatep", bufs=2 * tpb) as gatepool,
            tc.tile_pool(name="smallp", bufs=6 * tpb) as smallpool,
            tc.tile_pool(name="psum", bufs=8, space="PSUM") as psump,
        ):
            ones = constp.tile([1, P], dtype)
            nc.vector.memset(ones[:], 1.0)

            gw_sb = constp.tile([P, kchunks, e], dtype)
            for k in range(kchunks):
                nc.sync.dma_start(out=gw_sb[:, k, :], in_=gw_d[k * P : (k + 1) * P, :])
            gb_sb = constp.tile([1, e], dtype)
            nc.sync.dma_start(out=gb_sb[:], in_=gb_d[:])
            eb_sb = constp.tile([1, e, cols], dtype)
            nc.sync.dma_start(out=eb_sb[:], in_=eb_d[:])

            for b in range(nblocks):
                xts = []
                for k in range(kchunks):
                    xtile = xpool.tile([P, tb], dtype, tag="xb")
                    nc.sync.dma_start(
                        out=xtile[:],
                        in_=xt_d[k * P : (k + 1) * P, b * tb : (b + 1) * tb],
                    )
                    xts.append(xtile)

                # Gate softmax for each token tile of the block.
                gates = []
                for t in range(tpb):
                    pg = psump.tile([P, e], F32, tag="ps")
                    for k in range(kchunks):
                        nc.tensor.matmul(
                            pg[:],
                            xts[k][:, t * P : (t + 1) * P],
                            gw_sb[:, k, :],
                            start=(k == 0),
                            stop=False,
                        )
                    nc.tensor.matmul(pg[:], ones[:], gb_sb[:], start=False, stop=True)

                    negmax = smallpool.tile([P, 1], F32, tag="sm")
                    nc.vector.tensor_reduce(
                        out=negmax[:],
                        in_=pg[:],
                        axis=mybir.AxisListType.X,
                        op=mybir.AluOpType.max,
                        negate=True,
                    )
                    gexp = gatepool.tile([P, e], F32, tag="g")
                    sumexp = smallpool.tile([P, 1], F32, tag="sm")
                    nc.scalar.activation(
                        out=gexp[:],
                        in_=pg[:],
                        func=mybir.ActivationFunctionType.Exp,
                        bias=negmax[:],
                        scale=1.0,
                        accum_out=sumexp[:],
                    )
                    recip = smallpool.tile([P, 1], F32, tag="sm")
                    nc.vector.reciprocal(out=recip[:], in_=sumexp[:])
                    gate_sb = gatepool.tile([P, e], F32, tag="g")
                    nc.vector.tensor_scalar_mul(
                        out=gate_sb[:], in0=gexp[:], scalar1=recip[:]
                    )
                    gates.append(gate_sb)

                # Expert matmuls + gate-weighted combine.
                accs = [accpool.tile([P, cols], F32, tag="acc", name=f"acc_{b}_{t}") for t in range(tpb)]
                for ei in range(e):
                    pss = [psump.tile([P, cols], F32, tag="ps", name=f"ps_{b}_{ei}_{t}") for t in range(tpb)]
                    for k in range(kchunks):
                        wt = wpool.tile([P, cols], dtype, tag="w")
                        nc.sync.dma_start(
                            out=wt[:], in_=wc_d[ei, k * P : (k + 1) * P, :]
                        )
                        for t in range(tpb):
                            nc.tensor.matmul(
                                pss[t][:],
                                xts[k][:, t * P : (t + 1) * P],
                                wt[:],
                                start=(k == 0),
                                stop=False,
                            )
                    for t in range(tpb):
                        nc.tensor.matmul(
                            pss[t][:], ones[:], eb_sb[:, ei, :], start=False, stop=True
                        )
                        gcol = gates[t][:, ei : ei + 1]
                        if ei == 0:
                            nc.vector.tensor_scalar_mul(
                                out=accs[t][:], in0=pss[t][:], scalar1=gcol
                            )
                        else:
                            nc.vector.scalar_tensor_tensor(
                                out=accs[t][:],
                                in0=pss[t][:],
                                scalar=gcol,
                                in1=accs[t][:],
                                op0=mybir.AluOpType.mult,
                                op1=mybir.AluOpType.add,
                            )
                for t in range(tpb):
                    row0 = (b * tpb + t) * P
                    nc.sync.dma_start(out=out_d[row0 : row0 + P, :], in_=accs[t][:])

    nc.compile()
    return nc


_prog_cache = {}


def _get_program():
    if "nc" not in _prog_cache:
        _prog_cache["nc"] = build_moe_program()
    return _prog_cache["nc"]


def make_in_maps(x, gate_w, gate_b, expert_w, expert_b, n_cores=N_CORES, cols=COLS):
    bf16 = ml_dtypes.bfloat16
    e = expert_w.shape[0]
    xt = np.ascontiguousarray(np.asarray(x).T).astype(bf16)
    gw = np.asarray(gate_w).astype(bf16)
    gb = np.asarray(gate_b).reshape(1, e).astype(bf16)
    ew = np.asarray(expert_w)
    ebf = np.asarray(expert_b)
    in_maps = []
    for c in range(n_cores):
        sl = slice(c * cols, (c + 1) * cols)
        in_maps.append(
            {
                "xt": xt,
                "wc": np.ascontiguousarray(ew[:, :, sl]).astype(bf16),
                "gw": gw,
                "gb": gb,
                "eb": np.ascontiguousarray(ebf[:, sl]).reshape(1, e, cols).astype(bf16),
            }
        )
    return in_maps


def run_on_hw(nc, in_maps, **kwargs):
    old_m = nc.m
    nc.m = get_hw_module(nc.m)
    try:
        return run_bass_kernel_spmd(
            nc, in_maps, core_ids=list(range(len(in_maps))), **kwargs
        )
    finally:
        nc.m = old_m


def kernel(x, gate_w, gate_b, expert_w, expert_b):
    nc = _get_program()
    in_maps = make_in_maps(x, gate_w, gate_b, expert_w, expert_b)
    res = run_on_hw(nc, in_maps)
    out = np.concatenate([r["out"] for r in res.results], axis=1)
    return np.ascontiguousarray(out.astype(np.float32))


# revision 15
# speedup vs baseline: 6.3079x; 6.3079x over previous
"""Trainium2 Bass kernel for a dense MoE layer.

Reference computation (all experts run on every token):
    gate = softmax(x @ gate_w + gate_b)                       # [N, E]
    expert_out[e] = x @ expert_w[e] + expert_b[e]             # [E, N, R]
    out = einsum('ne,enr->nr', gate, expert_out)              # [N, R]

Sharding: output-column parallel across 8 cores. Core c computes
out[:, c*512:(c+1)*512] using the full x and the column slice of every
expert's weights. No collectives — the host concatenates the slices.

Device program (per core, SPMD identical, data differs):
  - x arrives pre-transposed (host) as xt[d_in, n_tok] so k-chunks load
    straight into matmul lhsT layout [K=128, M=128 tokens].
  - Loop over token blocks of TB tokens; x block resident in SBUF,
    expert weights stream (N_TOK/TB passes over the 32MB weight slice).
  - Gate: PE matmul accumulation into PSUM [128, E], bias added via a
    K=1 matmul against a ones vector, then max/exp/sum/normalize
    (softmax) on DVE+ACT.
  - Experts: for each expert, 32 K-chunk matmuls accumulate x @ W_e
    into a PSUM bank per token tile; expert bias added via K=1 matmul.
  - Combine: ScalarE (closest engine to PSUM) does the gate-scaled
    PSUM->SBUF copy, then DVE adds it into the accumulator
    out-of-place (ping-pong generations; in-place acc chains and
    PSUM-source DVE tensor-tensor ops both measured ~3x slower).

Measured on trn2 (R-delta method, axon wall-clock): ~2.3 ms/core
device time vs 1.75 ms bf16 PE roofline (~75%). Numerics: bf16
matmuls with fp32 PSUM accumulation -> ~3.1e-3 absmax-relative error
vs the fp32 reference.
"""

import numpy as np
import ml_dtypes

import concourse.bass as bass  # noqa: F401  (registers rust bindings)
import concourse.mybir as mybir
import concourse.tile as tile
from concourse import bacc
from concourse.bass_utils import run_bass_kernel_spmd
from concourse.bass_interp import get_hw_module

N_CORES = 8
N_TOK, D_IN, D_OUT, E = 4096, 4096, 4096, 8
COLS = D_OUT // N_CORES  # 512 output columns per core
P = 128
TB = 1024  # tokens per block resident in SBUF

F32 = mybir.dt.float32


def build_moe_program(
    n_tok=N_TOK,
    d_in=D_IN,
    cols=COLS,
    e=E,
    tb=TB,
    dtype=mybir.dt.bfloat16,
    repeat=1,
    do_gate=True,
    do_combine=True,
    combine_mode="act",
    w_batch=1,
    same_lhs=False,
):
    assert n_tok % tb == 0 and tb % P == 0 and d_in % P == 0
    kchunks = d_in // P
    tpb = tb // P  # token tiles per block
    nblocks = n_tok // tb

    nc = bacc.Bacc("TRN2", target_bir_lowering=False, debug=False)

    xt_d = nc.dram_tensor("xt", [d_in, n_tok], dtype, kind="ExternalInput")
    wc_d = nc.dram_tensor("wc", [e, d_in, cols], dtype, kind="ExternalInput")
    gw_d = nc.dram_tensor("gw", [d_in, e], dtype, kind="ExternalInput")
    gb_d = nc.dram_tensor("gb", [1, e], dtype, kind="ExternalInput")
    eb_d = nc.dram_tensor("eb", [1, e, cols], dtype, kind="ExternalInput")
    out_d = nc.dram_tensor("out", [n_tok, cols], F32, kind="ExternalOutput")

    with tile.TileContext(nc) as tc:
        with (
            tc.tile_pool(name="const", bufs=1) as constp,
            tc.tile_pool(name="xp", bufs=2 * kchunks) as xpool,
            tc.tile_pool(name="wp", bufs=(6 if w_batch == 1 else 3)) as wpool,
            tc.tile_pool(name="accp", bufs=2 * tpb) as accpool,
            tc.tile_pool(name="tmpp", bufs=4) as tmppool,
            tc.tile_pool(name="gatep", bufs=2 * tpb) as gatepool,
            tc.tile_pool(name="smallp", bufs=6 * tpb) as smallpool,
            tc.tile_pool(name="psum", bufs=8, space="PSUM") as psump,
        ):
            ones = constp.tile([1, P], dtype)
            nc.vector.memset(ones[:], 1.0)

            gw_sb = constp.tile([P, kchunks, e], dtype)
            for k in range(kchunks):
                nc.sync.dma_start(out=gw_sb[:, k, :], in_=gw_d[k * P : (k + 1) * P, :])
            gb_sb = constp.tile([1, e], dtype)
            nc.sync.dma_start(out=gb_sb[:], in_=gb_d[:])
            eb_sb = constp.tile([1, e, cols], dtype)
            nc.sync.dma_start(out=eb_sb[:], in_=eb_d[:])

            for rep in range(repeat):
              for b in range(nblocks):
                xts = []
                for k in range(kchunks):
                    xtile = xpool.tile([P, tb], dtype, tag="xb")
                    nc.sync.dma_start(
                        out=xtile[:],
                        in_=xt_d[k * P : (k + 1) * P, b * tb : (b + 1) * tb],
                    )
                    xts.append(xtile)

                # Gate softmax for each token tile of the block.
                gates = []
                for t in range(tpb if do_gate else 0):
                    pg = psump.tile([P, e], F32, tag="ps")
                    for k in range(kchunks):
                        nc.tensor.matmul(
                            pg[:],
                            xts[k][:, t * P : (t + 1) * P],
                            gw_sb[:, k, :],
                            start=(k == 0),
                            stop=False,
                        )
                    nc.tensor.matmul(pg[:], ones[:], gb_sb[:], start=False, stop=True)

                    negmax = smallpool.tile([P, 1], F32, tag="sm")
                    nc.vector.tensor_reduce(
                        out=negmax[:],
                        in_=pg[:],
                        axis=mybir.AxisListType.X,
                        op=mybir.AluOpType.max,
                        negate=True,
                    )
                    gexp = gatepool.tile([P, e], F32, tag="g")
                    sumexp = smallpool.tile([P, 1], F32, tag="sm")
                    nc.scalar.activation(
                        out=gexp[:],
                        in_=pg[:],
                        func=mybir.ActivationFunctionType.Exp,
                        bias=negmax[:],
                        scale=1.0,
                        accum_out=sumexp[:],
                    )
                    recip = smallpool.tile([P, 1], F32, tag="sm")
                    nc.vector.reciprocal(out=recip[:], in_=sumexp[:])
                    gate_sb = gatepool.tile([P, e], F32, tag="g")
                    nc.vector.tensor_scalar_mul(
                        out=gate_sb[:], in0=gexp[:], scalar1=recip[:]
                    )
                    gates.append(gate_sb)

                # Expert matmuls + gate-weighted combine.
                accs = [None] * tpb  # latest acc generation per token tile
                for ei in range(e):
                    pss = [psump.tile([P, cols], F32, tag="ps", name=f"ps_{b}_{ei}_{t}") for t in range(tpb)]
                    wview = wc_d[ei].rearrange("(kg p) c -> p kg c", p=P)
                    for kg in range(kchunks // w_batch):
                        wt = wpool.tile([P, w_batch, cols], dtype, tag="w")
                        nc.sync.dma_start(
                            out=wt[:],
                            in_=wview[:, kg * w_batch : (kg + 1) * w_batch, :],
                        )
                        for g in range(w_batch):
                            k = kg * w_batch + g
                            for t in range(tpb):
                                lhs_sl = (
                                    xts[0][:, 0:P]
                                    if same_lhs
                                    else xts[k][:, t * P : (t + 1) * P]
                                )
                                nc.tensor.matmul(
                                    pss[t][:],
                                    lhs_sl,
                                    wt[:, g, :],
                                    start=(k == 0),
                                    stop=False,
                                )
                    for t in range(tpb):
                        nc.tensor.matmul(
                            pss[t][:], ones[:], eb_sb[:, ei, :], start=False, stop=True
                        )
                        gcol = gates[t][:, ei : ei + 1] if do_gate else 0.125
                        if combine_mode == "act":
                            # ScalarE reads PSUM fast; DVE add stays SBUF-only
                            # and out-of-place (in-place acc chains serialize).
                            if ei == 0:
                                acc0 = accpool.tile(
                                    [P, cols], F32, tag="acc", name=f"acc_{b}_0_{t}"
                                )
                                nc.scalar.activation(
                                    out=acc0[:],
                                    in_=pss[t][:],
                                    func=mybir.ActivationFunctionType.Copy,
                                    scale=gcol,
                                )
                                accs[t] = acc0
                            elif do_combine:
                                tmp = tmppool.tile(
                                    [P, cols], F32, tag="tmp", name=f"tmp_{b}_{ei}_{t}"
                                )
                                nc.scalar.activation(
                                    out=tmp[:],
                                    in_=pss[t][:],
                                    func=mybir.ActivationFunctionType.Copy,
                                    scale=gcol,
                                )
                                nxt = accpool.tile(
                                    [P, cols], F32, tag="acc", name=f"acc_{b}_{ei}_{t}"
                                )
                                nc.vector.tensor_add(
                                    out=nxt[:], in0=tmp[:], in1=accs[t][:]
                                )
                                accs[t] = nxt
                        else:  # "stt": fused DVE combine straight from PSUM
                            if ei == 0:
                                acc0 = accpool.tile(
                                    [P, cols], F32, tag="acc", name=f"acc_{b}_0_{t}"
                                )
                                nc.vector.tensor_scalar_mul(
                                    out=acc0[:], in0=pss[t][:], scalar1=gcol
                                )
                                accs[t] = acc0
                            elif do_combine:
                                nc.vector.scalar_tensor_tensor(
                                    out=accs[t][:],
                                    in0=pss[t][:],
                                    scalar=gcol,
                                    in1=accs[t][:],
                                    op0=mybir.AluOpType.mult,
                                    op1=mybir.AluOpType.add,
                                )
                for t in range(tpb):
                    row0 = (b * tpb + t) * P
                    nc.sync.dma_start(out=out_d[row0 : row0 + P, :], in_=accs[t][:])

    nc.compile()
    return nc


_prog_cache = {}


def _get_program():
    if "nc" not in _prog_cache:
        _prog_cache["nc"] = build_moe_program()
    return _prog_cache["nc"]


def make_in_maps(x, gate_w, gate_b, expert_w, expert_b, n_cores=N_CORES, cols=COLS):
    bf16 = ml_dtypes.bfloat16
    e = expert_w.shape[0]
    xt = np.ascontiguousarray(np.asarray(x).T).astype(bf16)
    gw = np.asarray(gate_w).astype(bf16)
    gb = np.asarray(gate_b).reshape(1, e).astype(bf16)
    ew = np.asarray(expert_w)
    ebf = np.asarray(expert_b)
    in_maps = []
    for c in range(n_cores):
        sl = slice(c * cols, (c + 1) * cols)
        in_maps.append(
            {
                "xt": xt,
                "wc": np.ascontiguousarray(ew[:, :, sl]).astype(bf16),
                "gw": gw,
                "gb": gb,
                "eb": np.ascontiguousarray(ebf[:, sl]).reshape(1, e, cols).astype(bf16),
            }
        )
    return in_maps


def run_on_hw(nc, in_maps, **kwargs):
    old_m = nc.m
    nc.m = get_hw_module(nc.m)
    try:
        return run_bass_kernel_spmd(
            nc, in_maps, core_ids=list(range(len(in_maps))), **kwargs
        )
    finally:
        nc.m = old_m


def kernel(x, gate_w, gate_b, expert_w, expert_b):
    nc = _get_program()
    in_maps = make_in_maps(x, gate_w, gate_b, expert_w, expert_b)
    res = run_on_hw(nc, in_maps)
    out = np.concatenate([r["out"] for r in res.results], axis=1)
    return np.ascontiguousarray(out.astype(np.float32))


# revision 17
# speedup vs baseline: 6.6168x; 1.0490x over previous
"""Trainium2 Bass kernel for a dense MoE layer.

Reference computation (all experts run on every token):
    gate = softmax(x @ gate_w + gate_b)                       # [N, E]
    expert_out[e] = x @ expert_w[e] + expert_b[e]             # [E, N, R]
    out = einsum('ne,enr->nr', gate, expert_out)              # [N, R]

Sharding: output-column parallel across 8 cores. Core c computes
out[:, c*512:(c+1)*512] using the full x and the column slice of every
expert's weights. No collectives — the host concatenates the slices.

Device program (per core, SPMD identical, data differs):
  - x arrives pre-transposed (host) as xt[d_in, n_tok] so k-chunks load
    straight into matmul lhsT layout [K=128, M=128 tokens].
  - Loop over token blocks of TB tokens; x block resident in SBUF,
    expert weights stream (N_TOK/TB passes over the 32MB weight slice).
  - Gate: PE matmul accumulation into PSUM [128, E], bias added via a
    K=1 matmul against a ones vector, then max/exp/sum/normalize
    (softmax) on DVE+ACT.
  - Experts: for each expert, 32 K-chunk matmuls accumulate x @ W_e
    into a PSUM bank per token tile; expert bias added via K=1 matmul.
  - Combine: ScalarE (closest engine to PSUM) does the gate-scaled
    PSUM->SBUF copy, then DVE adds it into the accumulator
    out-of-place (ping-pong generations; in-place acc chains and
    PSUM-source DVE tensor-tensor ops both measured ~3x slower).

Measured on trn2 (R-delta method, axon wall-clock): ~2.3 ms/core
device time vs 1.75 ms bf16 PE roofline (~75%). Numerics: bf16
matmuls with fp32 PSUM accumulation -> ~3.1e-3 absmax-relative error
vs the fp32 reference.
"""

import numpy as np
import ml_dtypes

import concourse.bass as bass  # noqa: F401  (registers rust bindings)
import concourse.mybir as mybir
import concourse.tile as tile
from concourse import bacc
from concourse.bass_utils import run_bass_kernel_spmd
from concourse.bass_interp import get_hw_module

N_CORES = 8
N_TOK, D_IN, D_OUT, E = 4096, 4096, 4096, 8
COLS = D_OUT // N_CORES  # 512 output columns per core
P = 128
TB = 1024  # tokens per block resident in SBUF

F32 = mybir.dt.float32


def build_moe_program(
    n_tok=N_TOK,
    d_in=D_IN,
    cols=COLS,
    e=E,
    tb=TB,
    dtype=mybir.dt.bfloat16,
    repeat=1,
    do_gate=True,
    do_combine=True,
    combine_mode="act",
    w_batch=1,
    same_lhs=False,
    with_bias=True,
):
    assert n_tok % tb == 0 and tb % P == 0 and d_in % P == 0
    kchunks = d_in // P
    tpb = tb // P  # token tiles per block
    nblocks = n_tok // tb

    nc = bacc.Bacc("TRN2", target_bir_lowering=False, debug=False)

    xt_d = nc.dram_tensor("xt", [d_in, n_tok], dtype, kind="ExternalInput")
    wc_d = nc.dram_tensor("wc", [e, d_in, cols], dtype, kind="ExternalInput")
    gw_d = nc.dram_tensor("gw", [d_in, e], dtype, kind="ExternalInput")
    gb_d = nc.dram_tensor("gb", [1, e], dtype, kind="ExternalInput")
    eb_d = nc.dram_tensor("eb", [1, e, cols], dtype, kind="ExternalInput")
    out_d = nc.dram_tensor("out", [n_tok, cols], F32, kind="ExternalOutput")

    with tile.TileContext(nc) as tc:
        with (
            tc.tile_pool(name="const", bufs=1) as constp,
            tc.tile_pool(name="xp", bufs=2 * kchunks) as xpool,
            tc.tile_pool(name="wp", bufs=(6 if w_batch == 1 else 3)) as wpool,
            tc.tile_pool(name="accp", bufs=2 * tpb) as accpool,
            tc.tile_pool(name="tmpp", bufs=4) as tmppool,
            tc.tile_pool(name="gatep", bufs=2 * tpb) as gatepool,
            tc.tile_pool(name="smallp", bufs=6 * tpb) as smallpool,
            tc.tile_pool(name="psum", bufs=8, space="PSUM") as psump,
        ):
            ones = constp.tile([1, P], dtype)
            nc.vector.memset(ones[:], 1.0)

            gw_sb = constp.tile([P, kchunks, e], dtype)
            for k in range(kchunks):
                nc.sync.dma_start(out=gw_sb[:, k, :], in_=gw_d[k * P : (k + 1) * P, :])
            gb_sb = constp.tile([1, e], dtype)
            nc.sync.dma_start(out=gb_sb[:], in_=gb_d[:])
            eb_sb = constp.tile([1, e, cols], dtype)
            nc.sync.dma_start(out=eb_sb[:], in_=eb_d[:])

            for rep in range(repeat):
              for b in range(nblocks):
                xts = []
                for k in range(kchunks):
                    xtile = xpool.tile([P, tb], dtype, tag="xb")
                    nc.sync.dma_start(
                        out=xtile[:],
                        in_=xt_d[k * P : (k + 1) * P, b * tb : (b + 1) * tb],
                    )
                    xts.append(xtile)

                # Gate softmax for each token tile of the block.
                gates = []
                for t in range(tpb if do_gate else 0):
                    pg = psump.tile([P, e], F32, tag="ps")
                    for k in range(kchunks):
                        nc.tensor.matmul(
                            pg[:],
                            xts[k][:, t * P : (t + 1) * P],
                            gw_sb[:, k, :],
                            start=(k == 0),
                            stop=(not with_bias and k == kchunks - 1),
                        )
                    if with_bias:
                        nc.tensor.matmul(
                            pg[:], ones[:], gb_sb[:], start=False, stop=True
                        )

                    negmax = smallpool.tile([P, 1], F32, tag="sm")
                    nc.vector.tensor_reduce(
                        out=negmax[:],
                        in_=pg[:],
                        axis=mybir.AxisListType.X,
                        op=mybir.AluOpType.max,
                        negate=True,
                    )
                    gexp = gatepool.tile([P, e], F32, tag="g")
                    sumexp = smallpool.tile([P, 1], F32, tag="sm")
                    nc.scalar.activation(
                        out=gexp[:],
                        in_=pg[:],
                        func=mybir.ActivationFunctionType.Exp,
                        bias=negmax[:],
                        scale=1.0,
                        accum_out=sumexp[:],
                    )
                    recip = smallpool.tile([P, 1], F32, tag="sm")
                    nc.vector.reciprocal(out=recip[:], in_=sumexp[:])
                    gate_sb = gatepool.tile([P, e], F32, tag="g")
                    nc.vector.tensor_scalar_mul(
                        out=gate_sb[:], in0=gexp[:], scalar1=recip[:]
                    )
                    gates.append(gate_sb)

                # Expert matmuls + gate-weighted combine.
                accs = [None] * tpb  # latest acc generation per token tile
                for ei in range(e):
                    pss = [psump.tile([P, cols], F32, tag="ps", name=f"ps_{b}_{ei}_{t}") for t in range(tpb)]
                    wview = wc_d[ei].rearrange("(kg p) c -> p kg c", p=P)
                    for kg in range(kchunks // w_batch):
                        wt = wpool.tile([P, w_batch, cols], dtype, tag="w")
                        nc.sync.dma_start(
                            out=wt[:],
                            in_=wview[:, kg * w_batch : (kg + 1) * w_batch, :],
                        )
                        for g in range(w_batch):
                            k = kg * w_batch + g
                            for t in range(tpb):
                                lhs_sl = (
                                    xts[0][:, 0:P]
                                    if same_lhs
                                    else xts[k][:, t * P : (t + 1) * P]
                                )
                                nc.tensor.matmul(
                                    pss[t][:],
                                    lhs_sl,
                                    wt[:, g, :],
                                    start=(k == 0),
                                    stop=(not with_bias and k == kchunks - 1),
                                )
                    for t in range(tpb):
                        if with_bias:
                            nc.tensor.matmul(
                                pss[t][:],
                                ones[:],
                                eb_sb[:, ei, :],
                                start=False,
                                stop=True,
                            )
                        gcol = gates[t][:, ei : ei + 1] if do_gate else 0.125
                        if combine_mode == "act":
                            # ScalarE reads PSUM fast; DVE add stays SBUF-only
                            # and out-of-place (in-place acc chains serialize).
                            if ei == 0:
                                acc0 = accpool.tile(
                                    [P, cols], F32, tag="acc", name=f"acc_{b}_0_{t}"
                                )
                                nc.scalar.activation(
                                    out=acc0[:],
                                    in_=pss[t][:],
                                    func=mybir.ActivationFunctionType.Copy,
                                    scale=gcol,
                                )
                                accs[t] = acc0
                            elif do_combine:
                                tmp = tmppool.tile(
                                    [P, cols], F32, tag="tmp", name=f"tmp_{b}_{ei}_{t}"
                                )
                                nc.scalar.activation(
                                    out=tmp[:],
                                    in_=pss[t][:],
                                    func=mybir.ActivationFunctionType.Copy,
                                    scale=gcol,
                                )
                                nxt = accpool.tile(
                                    [P, cols], F32, tag="acc", name=f"acc_{b}_{ei}_{t}"
                                )
                                nc.vector.tensor_add(
                                    out=nxt[:], in0=tmp[:], in1=accs[t][:]
                                )
                                accs[t] = nxt
                        else:  # "stt": fused DVE combine straight from PSUM
                            if ei == 0:
                                acc0 = accpool.tile(
                                    [P, cols], F32, tag="acc", name=f"acc_{b}_0_{t}"
                                )
                                nc.vector.tensor_scalar_mul(
                                    out=acc0[:], in0=pss[t][:], scalar1=gcol
                                )
                                accs[t] = acc0
                            elif do_combine:
                                nc.vector.scalar_tensor_tensor(
                                    out=accs[t][:],
                                    in0=pss[t][:],
                                    scalar=gcol,
                                    in1=accs[t][:],
                                    op0=mybir.AluOpType.mult,
                                    op1=mybir.AluOpType.add,
                                )
                for t in range(tpb):
                    row0 = (b * tpb + t) * P
                    nc.sync.dma_start(out=out_d[row0 : row0 + P, :], in_=accs[t][:])

    nc.compile()
    return nc


_prog_cache = {}


def _get_program(with_bias=True):
    key = ("nc", with_bias)
    if key not in _prog_cache:
        _prog_cache[key] = build_moe_program(with_bias=with_bias)
    return _prog_cache[key]


def make_in_maps(x, gate_w, gate_b, expert_w, expert_b, n_cores=N_CORES, cols=COLS):
    bf16 = ml_dtypes.bfloat16
    e = expert_w.shape[0]
    xt = np.ascontiguousarray(np.asarray(x).T).astype(bf16)
    gw = np.asarray(gate_w).astype(bf16)
    gb = np.asarray(gate_b).reshape(1, e).astype(bf16)
    ew = np.asarray(expert_w)
    ebf = np.asarray(expert_b)
    in_maps = []
    for c in range(n_cores):
        sl = slice(c * cols, (c + 1) * cols)
        in_maps.append(
            {
                "xt": xt,
                "wc": np.ascontiguousarray(ew[:, :, sl]).astype(bf16),
                "gw": gw,
                "gb": gb,
                "eb": np.ascontiguousarray(ebf[:, sl]).reshape(1, e, cols).astype(bf16),
            }
        )
    return in_maps


def run_on_hw(nc, in_maps, **kwargs):
    old_m = nc.m
    nc.m = get_hw_module(nc.m)
    try:
        return run_bass_kernel_spmd(
            nc, in_maps, core_ids=list(range(len(in_maps))), **kwargs
        )
    finally:
        nc.m = old_m


def kernel(x, gate_w, gate_b, expert_w, expert_b):
    with_bias = bool(np.any(np.asarray(gate_b)) or np.any(np.asarray(expert_b)))
    nc = _get_program(with_bias=with_bias)
    in_maps = make_in_maps(x, gate_w, gate_b, expert_w, expert_b)
    res = run_on_hw(nc, in_maps)
    out = np.concatenate([r["out"] for r in res.results], axis=1)
    return np.ascontiguousarray(out.astype(np.float32))
